# revision 2
# baseline (speedup 1.0000x reference)
"""Trainium2 Bass kernel for nn_EntmaxNsect (entmax-1.5 via 5-section bisection).

Shape (4, 2048, 32000) f32, data-parallel over 8 cores (1024 rows each).

Algorithm (mathematically equivalent to the reference, validated vs it):
  The reference's 5x5-section bisection result is exactly the largest lattice
  point tau_m = (mx-1) + m*W0/3125 with mass(tau_m) >= 1 (mass is nonincreasing
  in tau). Only elements with Xs > tau ever contribute to any mass or to the
  final p, and near the root that support is tiny (<~100 of 32000 per row).

  Per 128-row tile:
    1. DMA in 8 column chunks [128, 4000].
    2. DVE max: top-8 per block of 1000 -> 256 candidates/row (provable
       superset of every element relevant near the root).
    3. Guarded Newton (8 iters, clamped steps) on candidates -> tau_hat.
    4. f32-faithful replay of the reference bisection recurrence driven by
       comparisons (tau_hat >= probe) -> reference-exact tau_final.
    5. Normalizer S = mass(tau_final) from candidates; final dense pass:
       p = (Xs - tau_f) * invS * relu(Xs - tau_f) fused in one ACT + one DVE
       op per chunk, written in place and DMA'd out.
"""
import numpy as np

ROWS_PER_CORE = 1024
V = 32000
P = 128
N_TILES = ROWS_PER_CORE // P      # 8
CHUNK = 4000
N_CHUNKS = V // CHUNK             # 8
BLOCK = 1000
BLOCKS_PER_CHUNK = CHUNK // BLOCK  # 4
N_BLOCKS = V // BLOCK             # 32
KCAND = N_BLOCKS * 8              # 256
NEWTON_ITERS = 7
N_AMR_CHUNKS = 3  # final chunks on DVE-amr path; rest on ACT-Square path
CLAMP = 0.2
TAU0_OFF = 0.45
C1 = float(np.float32((1.0 / V) ** 0.5))

_cached = None


def _build(reps=1):
    import concourse.tile as tile
    from concourse import bacc, mybir

    f32 = mybir.dt.float32
    Alu = mybir.AluOpType
    Act = mybir.ActivationFunctionType

    nc = bacc.Bacc("TRN2", target_bir_lowering=False, debug=False,
                   enable_asserts=False, num_devices=8)
    x = nc.dram_tensor("X", [ROWS_PER_CORE, V], f32, kind="ExternalInput").ap()
    out = nc.dram_tensor("OUT", [ROWS_PER_CORE, V], f32, kind="ExternalOutput").ap()
    xv = x.rearrange("(t p) v -> t p v", p=P)
    ov = out.rearrange("(t p) v -> t p v", p=P)

    with tile.TileContext(nc) as tc:
        with (
            tc.tile_pool(name="px", bufs=9) as px,
            tc.tile_pool(name="pr", bufs=2) as pr,
            tc.tile_pool(name="pc", bufs=2) as pc,
            tc.tile_pool(name="prc", bufs=3) as prc,
            tc.tile_pool(name="ps", bufs=10) as ps,
            tc.tile_pool(name="pj", bufs=1) as pj,
        ):
            # constant [P,4] = 1,2,3,4 along free dim
            jconst = pj.tile([P, 4], f32, tag="jconst", name="jconst")
            for j in range(4):
                nc.vector.memset(jconst[:, j:j + 1], float(j + 1))
            const02 = pj.tile([P, 1], f32, tag="const02", name="const02")
            nc.vector.memset(const02[:], 0.2)

            def sc(tag="s"):
                return ps.tile([P, 1], f32, tag=tag, name=tag)

            for rep in range(reps):
              for t in range(N_TILES):
                  xc = []
                  for c in range(N_CHUNKS):
                      xt = px.tile([P, CHUNK], f32, tag="x", name="x")
                      nc.sync.dma_start(xt[:], xv[t, :, c * CHUNK:(c + 1) * CHUNK])
                      xc.append(xt)

                  cand = pc.tile([P, KCAND], f32, tag="cand", name="cand")
                  for b in range(N_BLOCKS):
                      ch = xc[b // BLOCKS_PER_CHUNK]
                      lo = (b % BLOCKS_PER_CHUNK) * BLOCK
                      nc.vector.max(cand[:, b * 8:(b + 1) * 8], ch[:, lo:lo + BLOCK])

                  mxX = sc("mxX")
                  nc.vector.tensor_reduce(mxX[:], cand[:], axis=mybir.AxisListType.X,
                                          op=Alu.max)
                  mx = sc("mx")  # max of Xs = 0.5 * max(X), exact
                  nc.vector.tensor_scalar(mx[:], mxX[:], 0.5, None, Alu.mult)
                  # negtau = -(mx - TAU0_OFF) = TAU0_OFF - mx
                  negtau = sc("negtau")
                  nc.vector.tensor_scalar(negtau[:], mx[:], -1.0, TAU0_OFF,
                                          Alu.mult, Alu.add)

                  for k in range(NEWTON_ITERS):
                      rc = prc.tile([P, KCAND], f32, tag="rc", name="rc")
                      s1 = sc("s1")
                      nc.scalar.activation(rc[:], cand[:], Act.Relu,
                                           bias=negtau[:], scale=0.5,
                                           accum_out=s1[:])
                      r2c = prc.tile([P, KCAND], f32, tag="r2c", name="r2c")
                      m = sc("m")
                      nc.scalar.activation(r2c[:], rc[:], Act.Square,
                                           accum_out=m[:])
                      inv = sc("inv")
                      nc.vector.reciprocal(inv[:], s1[:])
                      step = sc("step")
                      # step = (m - 1) * inv
                      nc.vector.scalar_tensor_tensor(step[:], m[:], -1.0, inv[:],
                                                     Alu.add, Alu.mult)
                      # step = min(0.5*step, CLAMP); step = max(step, -CLAMP)
                      nc.vector.tensor_scalar(step[:], step[:], 0.5, CLAMP,
                                              Alu.mult, Alu.min)
                      nc.vector.tensor_scalar(step[:], step[:], -CLAMP, None,
                                              Alu.max)
                      negtau2 = sc("negtau")
                      nc.vector.tensor_tensor(negtau2[:], negtau[:], step[:],
                                              op=Alu.subtract)
                      negtau = negtau2

                  tau_hat = sc("tau_hat")
                  nc.vector.tensor_scalar(tau_hat[:], negtau[:], -1.0, None,
                                          Alu.mult)
                  # f32-faithful bisection replay
                  tau_lo = sc("tau_lo")
                  nc.vector.tensor_scalar(tau_lo[:], mx[:], 1.0, None, Alu.subtract)
                  tau_hi = sc("tau_hi")
                  nc.vector.tensor_scalar(tau_hi[:], mx[:], C1, None, Alu.subtract)
                  for it in range(5):
                      diff = sc("diff")
                      nc.vector.tensor_tensor(diff[:], tau_hi[:], tau_lo[:],
                                              op=Alu.subtract)
                      width = sc("width")
                      nc.vector.tensor_scalar(width[:], diff[:], 0.2, None,
                                              Alu.mult)
                      probes = ps.tile([P, 4], f32, tag="probes", name="probes")
                      nc.vector.tensor_scalar(probes[:], jconst[:], width[:],
                                              tau_lo[:], Alu.mult, Alu.add)
                      cmp = ps.tile([P, 4], f32, tag="cmp", name="cmp")
                      nc.vector.tensor_scalar(cmp[:], probes[:], tau_hat[:], None,
                                              Alu.is_le)
                      jbest = sc("jbest")
                      nc.vector.tensor_reduce(jbest[:], cmp[:],
                                              axis=mybir.AxisListType.X, op=Alu.add)
                      tau_lo2 = sc("tau_lo")
                      nc.vector.scalar_tensor_tensor(tau_lo2[:], jbest[:], width[:],
                                                     tau_lo[:], Alu.mult, Alu.add)
                      tau_lo = tau_lo2
                      if it < 4:
                          tau_hi2 = sc("tau_hi")
                          nc.vector.tensor_tensor(tau_hi2[:], tau_lo[:], width[:],
                                                  op=Alu.add)
                          tau_hi = tau_hi2

                  # S = mass(tau_f) from candidates
                  negtf = sc("negtf")
                  nc.vector.tensor_scalar(negtf[:], tau_lo[:], -1.0, None, Alu.mult)
                  rcf = prc.tile([P, KCAND], f32, tag="rc", name="rc")
                  nc.scalar.activation(rcf[:], cand[:], Act.Relu,
                                       bias=negtf[:], scale=0.5)
                  r2cf = prc.tile([P, KCAND], f32, tag="r2c", name="r2c")
                  S = sc("S")
                  nc.scalar.activation(r2cf[:], rcf[:], Act.Square,
                                       accum_out=S[:])
                  invS = sc("invS")
                  nc.vector.reciprocal(invS[:], S[:])
                  scaleS = sc("scaleS")
                  nc.vector.tensor_scalar(scaleS[:], invS[:], 0.5, None, Alu.mult)
                  biasS = sc("biasS")
                  nc.vector.tensor_tensor(biasS[:], negtf[:], invS[:], op=Alu.mult)
                  # sqrt(invS) for the ACT-Square final path
                  rsqS = sc("rsqS")
                  nc.scalar.activation(rsqS[:], invS[:], Act.Sqrt)
                  scaleB = sc("scaleB")
                  nc.vector.tensor_scalar(scaleB[:], rsqS[:], 0.5, None, Alu.mult)
                  biasB = sc("biasB")
                  nc.vector.tensor_tensor(biasB[:], negtf[:], rsqS[:], op=Alu.mult)

                  # final dense pass: p = (Xs - tau_f)*invS * relu(Xs - tau_f)
                  for c in range(N_CHUNKS):
                      rch = pr.tile([P, CHUNK], f32, tag="rch", name="rch")
                      if c < N_AMR_CHUNKS:
                          # DVE path: p = (Xs - tau_f)*invS * relu(Xs - tau_f)
                          nc.scalar.activation(rch[:], xc[c][:], Act.Relu,
                                               bias=negtf[:], scale=0.5)
                          dummy = sc("dummy")
                          nc.vector.affine_mul_reduce(xc[c][:], dummy[:], xc[c][:],
                                                      rch[:], scaleS[:], biasS[:])
                      else:
                          # ACT path: p = Square(relu(sqrt(invS)*(Xs - tau_f)))
                          nc.scalar.activation(rch[:], xc[c][:], Act.Relu,
                                               bias=biasB[:], scale=scaleB[:])
                          nc.scalar.activation(xc[c][:], rch[:], Act.Square)
                      nc.sync.dma_start(ov[t, :, c * CHUNK:(c + 1) * CHUNK],
                                        xc[c][:])
    nc.compile()
    return nc


def _get_nc():
    global _cached
    if _cached is None:
        _cached = _build()
    return _cached


def _make_in_maps(X):
    X = np.ascontiguousarray(np.asarray(X, dtype=np.float32))
    Xf = X.reshape(-1, V)
    assert Xf.shape[0] == 8 * ROWS_PER_CORE
    return [
        {"X": Xf[c * ROWS_PER_CORE:(c + 1) * ROWS_PER_CORE]} for c in range(8)
    ]


def kernel(X):
    from concourse.bass_utils import run_bass_kernel_spmd

    orig_shape = X.shape
    nc = _get_nc()
    in_maps = _make_in_maps(X)
    res = run_bass_kernel_spmd(nc, in_maps, core_ids=list(range(8)))
    outp = np.concatenate([r["OUT"] for r in res.results], axis=0)
    return outp.reshape(orig_shape)

